# revision 6
# baseline (speedup 1.0000x reference)
"""Trainium2 Bass kernel for nn_CompressiveMemory_57750130262084.

The reference computes (B=8, S=4096, DK=DV=1024):
    sigma  = elu(query) + 1                                  [B,S,DK]
    memory = einsum('bkd,bsv->bkv', swap(sigma), value)      [B,DK,DV]
    z_norm = sum_s sigma                                     [B,DK]
    out    = einsum('bsd,bkv->bsv', sigma, memory)
           / einsum('bsd,bk->bs',  sigma, z_norm)[..., None]

Every einsum uses disjoint summed subscripts, so each factorises into
outer products of independent reductions; everything cancels except
    out[b,s,v] = sum_s value[b,s,v]     (exactly; query cancels)

So the kernel is a column-sum of `value` over S, broadcast over S.
Sharding: data-parallel over batch, one NeuronCore per batch element.
Per-core work: read 16 MB, reduce 4096 rows -> 1 row, write 16 MB.

Measured facts driving this schedule (NTFF traces on this pod):
  - The 16 SDMA engines are ~100%% busy through both phases at ~line
    rate per 4 KB packet (~146 ns read / ~162 ns write) EXCEPT engine
    15, which runs ~14%% slower (166/185 ns) and straggles each phase
    by ~6 us.  HWDGE assigns descriptors to engines by partition//8,
    so engine 15 serves partitions 120-127.
  - A [120, X] DMA (partitions 0-119) spreads evenly over engines
    0-14 and gives engine 15 nothing.  (Odd partition counts like 92,
    and 2D-partition APs, degenerate onto 4 engines - measured.)
  - Row->partition assignment is arbitrary for a full sum, and all
    output rows are identical, so BOTH phases can be rebalanced:
    17 full [128]-row slots + 16 [120]-row slots = exactly 4096 rows.
    Engine 15 then carries 17 rows/partition vs 33 for the rest:
    every engine finishes its phase at ~38.5 us (read) / ~42.8 (write)
    instead of engine 15 trailing to 47.9 / 53.
  - f32 matmul = 2 HW passes per instruction: a [128,1024] chunk is
    ~1.7 us on the PE, ~1.23 us on the DVE; line rate is ~1.17
    us/slot, so the DVE takes most slots and the PE takes a few
    mid-stream ones plus the last slot and the accumulator fold.
  - DMA completion semaphores fire ~2.5 us after the last byte, so
    every DMA is small (480-512 KB) to keep consumers close behind.

Schedule per core:
  - 33 read DMAs on the sync HWDGE queue, alternating F=[128,1024] /
    T=[120,1024]; DVE chains 30 slots into acc, PE matmul-reduces 2
    mid-stream slots + the final slot into PSUM and folds acc in last
    (ones^T matmuls; T slots use ones[0:120]).
  - PSUM -> SBUF copy in halves (DVE + ACT in parallel); ACT table is
    preloaded by a dummy scalar.copy at t=0 (lazy load costs 1.3 us).
  - 6 write DMAs on the scalar HWDGE queue (separate logical queue)
    with step-0 broadcast source APs, 3x[128-partition] reps +
    3x[120-partition] reps mirroring the same engine rebalance.
"""

import numpy as np

B, S, D = 8, 4096, 1024
P = 128                 # SBUF partitions
H = 512                 # PSUM bank width in f32 (matmul N limit)
TP = 120                # partitions per T-slot (engines 0-14 only)
N_F, N_T = 17, 16       # 17*128 + 16*120 = 4096 rows exactly
PE_SLOTS = {23, 27, 32}  # slot positions reduced on the PE (rest DVE)

W_F = [6, 6, 5]         # reps per full-width write DMA  (sum = 17)
W_T = [6, 5, 5]         # reps per 120-partition write DMA (sum = 16)

_CACHE: dict = {}


def _build_program():
    import concourse.mybir as mybir
    import concourse.tile as tile
    from concourse import bacc

    f32 = mybir.dt.float32
    nc = bacc.Bacc("TRN2", target_bir_lowering=False, debug=False, num_devices=B, enable_asserts=False)
    v = nc.declare_dram_parameter("value", [S, D], f32, isOutput=False)
    o = nc.declare_dram_parameter("out", [S, D], f32, isOutput=True)
    vf, of = v[:], o[:]

    # slot sequence: F at even positions (17), T at odd (16); last = F.
    kinds = []
    for i in range(N_F + N_T):
        kinds.append("F" if i % 2 == 0 else "T")
    assert kinds.count("F") == N_F and kinds.count("T") == N_T

    with tile.TileContext(nc) as tcx:
        with (
            tcx.tile_pool(name="in", bufs=1) as in_pool,
            tcx.tile_pool(name="acc", bufs=1) as acc_pool,
            tcx.tile_pool(name="ones", bufs=1) as ones_pool,
            tcx.tile_pool(name="bcast", bufs=1) as bcast_pool,
            tcx.tile_pool(name="warm", bufs=1) as warm_pool,
            tcx.tile_pool(name="psum", bufs=1, space="PSUM") as psum_pool,
        ):
            # Preload the ACT table so the tail-time scalar.copy is cheap.
            warm = warm_pool.tile([P, 2], f32)
            nc.vector.memset(warm[:], 0.0)
            nc.scalar.copy(warm[:, 0:1], warm[:, 1:2])

            ones = ones_pool.tile([P, P], f32)
            nc.vector.memset(ones[:], 1.0)

            # ---- input DMAs (sync queue), one small DMA per slot.
            tiles = []
            row = 0
            for i, kind in enumerate(kinds):
                np_ = P if kind == "F" else TP
                t = in_pool.tile([np_, D], f32, tag=f"s{i}")
                nc.sync.dma_start(t[:], vf[row : row + np_])
                tiles.append((t, np_))
                row += np_
            assert row == S

            ps = psum_pool.tile([P, D], f32)

            def mm(moving, np_, start, stop):
                for h in range(2):
                    nc.tensor.matmul(
                        ps[:, h * H : (h + 1) * H],
                        ones[0:np_],
                        moving[0:np_, h * H : (h + 1) * H],
                        start=start,
                        stop=stop,
                    )

            # PE: two mid-stream slots (keeps the DVE ahead of the line
            # rate), emitted first so start=True lands there.
            mid_pe = sorted(PE_SLOTS)[:-1]
            for j, i in enumerate(mid_pe):
                t, np_ = tiles[i]
                mm(t[:], np_, start=(j == 0), stop=False)

            # DVE: everything else, chained into acc.
            dve = [i for i in range(len(kinds)) if i not in PE_SLOTS]
            acc = acc_pool.tile([P, D], f32)
            t0, np0 = tiles[dve[0]]
            assert np0 == P
            nc.vector.tensor_copy(acc[:], t0[:])
            for i in dve[1:]:
                t, np_ = tiles[i]
                nc.vector.tensor_add(acc[0:np_], acc[0:np_], t[:])

            # PE tail: final slot, then fold acc (stop=True).
            t_last, np_last = tiles[max(PE_SLOTS)]
            mm(t_last[:], np_last, start=False, stop=False)
            mm(acc, P, start=False, stop=True)

            # PSUM -> SBUF in parallel halves (DVE + ACT).
            bc = bcast_pool.tile([P, D], f32)
            nc.vector.tensor_copy(bc[:, 0:H], ps[:, 0:H])
            nc.scalar.copy(bc[:, H:D], ps[:, H:D])

            # ---- output DMAs (scalar queue), broadcast source, same
            # engine rebalance: F groups use 128 partitions, T groups 120.
            row = 0
            wf, wt = list(W_F), list(W_T)
            while wf or wt:
                if wf:
                    r = wf.pop(0)
                    dst = of[row : row + r * P].rearrange("(n p) m -> p n m", p=P)
                    nc.scalar.dma_start(
                        dst, bc[:].unsqueeze(1).to_broadcast((P, r, D))
                    )
                    row += r * P
                if wt:
                    r = wt.pop(0)
                    dst = of[row : row + r * TP].rearrange("(n p) m -> p n m", p=TP)
                    nc.scalar.dma_start(
                        dst, bc[0:TP].unsqueeze(1).to_broadcast((TP, r, D))
                    )
                    row += r * TP
            assert row == S

    nc.compile()
    return nc


def _get_program():
    if "nc" not in _CACHE:
        _CACHE["nc"] = _build_program()
    return _CACHE["nc"]


def kernel(query: np.ndarray, value: np.ndarray) -> np.ndarray:
    from concourse.bass_utils import run_bass_kernel_spmd

    del query  # output is exactly independent of query (see module docstring)
    value = np.ascontiguousarray(value, dtype=np.float32)
    assert value.shape == (B, S, D)

    nc = _get_program()
    in_maps = [{"value": value[b]} for b in range(B)]
    try:
        res = run_bass_kernel_spmd(nc, in_maps, list(range(B)))
    except Exception:
        # The tunneled runtime occasionally surfaces a transient
        # NRT_EXEC_UNIT_UNRECOVERABLE on the first dispatch; retry once.
        import time

        time.sleep(2.0)
        res = run_bass_kernel_spmd(nc, in_maps, list(range(B)))
    return np.stack([res.results[b]["out"] for b in range(B)], axis=0)


# revision 7
# speedup vs baseline: 1.3102x; 1.3102x over previous
"""Trainium2 Bass kernel for nn_CompressiveMemory_57750130262084.

The reference computes (B=8, S=4096, DK=DV=1024):
    sigma  = elu(query) + 1                                  [B,S,DK]
    memory = einsum('bkd,bsv->bkv', swap(sigma), value)      [B,DK,DV]
    z_norm = sum_s sigma                                     [B,DK]
    out    = einsum('bsd,bkv->bsv', sigma, memory)
           / einsum('bsd,bk->bs',  sigma, z_norm)[..., None]

Every einsum uses disjoint summed subscripts, so each factorises into
outer products of independent reductions; everything cancels except
    out[b,s,v] = sum_s value[b,s,v]     (exactly; query cancels)

So the kernel is a column-sum of `value` over S, broadcast over S.
Sharding: data-parallel over batch, one NeuronCore per batch element.
Per-core work: read 16 MB, reduce 4096 rows -> 1 row, write 16 MB.

Measured facts driving this schedule (NTFF traces on this pod):
  - Only full 128-partition DMAs hit the fast descriptor path
    (~146 ns read / ~162 ns write per 4 KB packet, all 16 SDMA
    engines ~100%% busy).  Partial-partition DMAs ([120,X], [92,X],
    2D-partition APs) degrade EVERY packet in the stream to ~270 ns -
    measured, so no partition-level rebalancing is possible.
  - SDMA engine 15 is ~14%% slower than the rest and straggles each
    phase by ~5-6 us.  Structural; absorbed into the budget.
  - f32 matmul = 2 HW passes per instruction: a [128,1024] slot costs
    ~1.7 us on the PE vs ~1.23 us on the DVE; slot line rate is
    ~1.25 us.  Neither engine alone keeps pace with slack, so slots
    are split ~2:1 DVE:PE (the mix the baseline sustained 405 GB/s
    with), interleaved so neither engine ever backlogs.
  - DMA completion semaphores fire ~2.5 us after the last byte, and
    consumers wait on whole-DMA sems, so every read DMA is one
    512 KB slot: consumers trail the stream by one slot + receipt
    instead of a 4 MB batch.

Schedule per core:
  - 32 x 512 KB read DMAs on the sync HWDGE queue.
  - DVE chains 21 slots into acc; PE matmul-reduces 11 slots into
    PSUM (ones^T accumulating matmuls).  The last slot is DVE's, the
    acc fold into PSUM is the only work after it: tail after the last
    input semaphore is ~1.2 us (add) + ~1.7 (fold) + ~0.7 (copy).
  - PSUM -> SBUF copy in halves (DVE + ACT in parallel); the ACT
    table is preloaded by a dummy scalar.copy at t=0 (the lazy
    ACT_TABLE_LOAD costs 1.3 us on the critical path otherwise).
  - 4 x 4 MB broadcast write DMAs on the scalar HWDGE queue (separate
    logical queue from the reads), step-0 source AP fanning the
    [128,1024] colsum tile to all 4096 rows.
"""

import numpy as np

B, S, D = 8, 4096, 1024
P = 128                 # SBUF partitions
H = 512                 # PSUM bank width in f32 (matmul N limit)
N_SLOT = S // P         # 32 x [128,1024] slots
PE_SLOTS = tuple(range(2, 30, 3)) + (30,)   # 11 slots on the PE; rest DVE
OUT_REP = 8             # row-slots per output DMA -> 4 MB writes
N_OUT = N_SLOT // OUT_REP

_CACHE: dict = {}


def _build_program():
    import concourse.mybir as mybir
    import concourse.tile as tile
    from concourse import bacc

    f32 = mybir.dt.float32
    nc = bacc.Bacc("TRN2", target_bir_lowering=False, debug=False, num_devices=B, enable_asserts=False)
    v = nc.declare_dram_parameter("value", [S, D], f32, isOutput=False)
    o = nc.declare_dram_parameter("out", [S, D], f32, isOutput=True)
    vf, of = v[:], o[:]

    with tile.TileContext(nc) as tcx:
        with (
            tcx.tile_pool(name="in", bufs=1) as in_pool,
            tcx.tile_pool(name="acc", bufs=1) as acc_pool,
            tcx.tile_pool(name="ones", bufs=1) as ones_pool,
            tcx.tile_pool(name="bcast", bufs=1) as bcast_pool,
            tcx.tile_pool(name="warm", bufs=1) as warm_pool,
            tcx.tile_pool(name="psum", bufs=1, space="PSUM") as psum_pool,
        ):
            # Preload the ACT table so the tail-time scalar.copy is cheap.
            warm = warm_pool.tile([P, 2], f32)
            nc.vector.memset(warm[:], 0.0)
            nc.scalar.copy(warm[:, 0:1], warm[:, 1:2])

            ones = ones_pool.tile([P, P], f32)
            nc.vector.memset(ones[:], 1.0)

            # ---- input: one 512 KB full-width DMA per slot (sync queue).
            tiles = []
            for k in range(N_SLOT):
                t = in_pool.tile([P, D], f32, tag=f"s{k}")
                nc.sync.dma_start(t[:], vf[k * P : (k + 1) * P])
                tiles.append(t)

            ps = psum_pool.tile([P, D], f32)

            def mm(moving, start, stop):
                for h in range(2):
                    nc.tensor.matmul(
                        ps[:, h * H : (h + 1) * H],
                        ones[:],
                        moving[:, h * H : (h + 1) * H],
                        start=start,
                        stop=stop,
                    )

            # PE slots (interleaved so the PE never backlogs; ~1.7us per
            # slot at a ~3.7us assigned-arrival cadence).
            for j, k in enumerate(PE_SLOTS):
                mm(tiles[k][:], start=(j == 0), stop=False)

            # DVE slots, chained into acc; the final slot is DVE's.
            dve = [k for k in range(N_SLOT) if k not in PE_SLOTS]
            acc = acc_pool.tile([P, D], f32)
            nc.vector.tensor_copy(acc[:], tiles[dve[0]][:])
            for k in dve[1:]:
                nc.vector.tensor_add(acc[:], acc[:], tiles[k][:])

            # Fold acc into PSUM: the only post-stream PE work.
            mm(acc, start=False, stop=True)

            # PSUM -> SBUF in parallel halves (DVE + ACT).
            bc = bcast_pool.tile([P, D], f32)
            nc.vector.tensor_copy(bc[:, 0:H], ps[:, 0:H])
            nc.scalar.copy(bc[:, H:D], ps[:, H:D])

            # ---- output: broadcast writes on the scalar queue.
            o_re = of.rearrange("(i n p) m -> i p n m", i=N_OUT, n=OUT_REP, p=P)
            src = bc[:].unsqueeze(1).to_broadcast((P, OUT_REP, D))
            for i in range(N_OUT):
                nc.scalar.dma_start(o_re[i], src)

    nc.compile()
    return nc


def _get_program():
    if "nc" not in _CACHE:
        _CACHE["nc"] = _build_program()
    return _CACHE["nc"]


def kernel(query: np.ndarray, value: np.ndarray) -> np.ndarray:
    from concourse.bass_utils import run_bass_kernel_spmd

    del query  # output is exactly independent of query (see module docstring)
    value = np.ascontiguousarray(value, dtype=np.float32)
    assert value.shape == (B, S, D)

    nc = _get_program()
    in_maps = [{"value": value[b]} for b in range(B)]
    try:
        res = run_bass_kernel_spmd(nc, in_maps, list(range(B)))
    except Exception:
        # The tunneled runtime occasionally surfaces a transient
        # NRT_EXEC_UNIT_UNRECOVERABLE on the first dispatch; retry once.
        import time

        time.sleep(2.0)
        res = run_bass_kernel_spmd(nc, in_maps, list(range(B)))
    return np.stack([res.results[b]["out"] for b in range(B)], axis=0)


# revision 9
# speedup vs baseline: 1.4417x; 1.1003x over previous
"""Trainium2 Bass kernel for nn_CompressiveMemory_57750130262084.

The reference computes (B=8, S=4096, DK=DV=1024):
    sigma  = elu(query) + 1                                  [B,S,DK]
    memory = einsum('bkd,bsv->bkv', swap(sigma), value)      [B,DK,DV]
    z_norm = sum_s sigma                                     [B,DK]
    out    = einsum('bsd,bkv->bsv', sigma, memory)
           / einsum('bsd,bk->bs',  sigma, z_norm)[..., None]

Every einsum uses disjoint summed subscripts, so each factorises into
outer products of independent reductions; everything cancels except
    out[b,s,v] = sum_s value[b,s,v]     (exactly; query cancels)

So the kernel is a column-sum of `value` over S, broadcast over S.
Sharding: data-parallel over batch, one NeuronCore per batch element.
Per-core work: read 16 MB, reduce 4096 rows -> 1 row, write 16 MB.

Measured facts driving this schedule (NTFF traces on this pod):
  - Only full 128-partition DMAs hit the fast descriptor path
    (~146 ns read / ~162 ns write per 4 KB packet, all 16 SDMA
    engines ~100%% busy).  Partial-partition DMAs ([120,X], [92,X],
    2D-partition APs) degrade EVERY packet in the stream to ~270 ns -
    measured, so no partition-level rebalancing is possible.
  - SDMA engine 15 is ~14%% slower than the rest and straggles each
    phase by ~5-6 us.  Structural; absorbed into the budget.
  - f32 matmul = 2 HW passes per instruction: a [128,1024] slot costs
    ~1.7 us on the PE vs ~1.23 us on the DVE; slot line rate is
    ~1.25 us.  Neither engine alone keeps pace with slack, so slots
    are split ~2:1 DVE:PE (the mix the baseline sustained 405 GB/s
    with), interleaved so neither engine ever backlogs.
  - DMA completion semaphores fire ~2.5 us after the last byte, and
    consumers wait on whole-DMA sems, so every read DMA is one
    512 KB slot: consumers trail the stream by one slot + receipt
    instead of a 4 MB batch.

Schedule per core:
  - 32 x 512 KB read DMAs on the sync HWDGE queue.
  - DVE chains 21 slots into acc; PE matmul-reduces 11 slots into
    PSUM (ones^T accumulating matmuls).  The last slot is DVE's, the
    acc fold into PSUM is the only work after it: tail after the last
    input semaphore is ~1.2 us (add) + ~1.7 (fold) + ~0.7 (copy).
  - PSUM -> SBUF copy in halves (DVE + ACT in parallel); the ACT
    table is preloaded by a dummy scalar.copy at t=0 (the lazy
    ACT_TABLE_LOAD costs 1.3 us on the critical path otherwise).
  - 4 x 4 MB broadcast write DMAs on the scalar HWDGE queue (separate
    logical queue from the reads), step-0 source AP fanning the
    [128,1024] colsum tile to all 4096 rows.
"""

import numpy as np

B, S, D = 8, 4096, 1024
P = 128                 # SBUF partitions
H = 512                 # PSUM bank width in f32 (matmul N limit)
N_SLOT = S // P         # 32 x [128,1024] slots
PE_SLOTS = tuple(range(3, 28, 3))   # 9 mid-stream slots on the PE; rest DVE
OUT_REP = 8             # row-slots per output DMA -> 4 MB writes
N_OUT = N_SLOT // OUT_REP

_CACHE: dict = {}


def _build_program():
    import concourse.mybir as mybir
    import concourse.tile as tile
    from concourse import bacc

    f32 = mybir.dt.float32
    nc = bacc.Bacc("TRN2", target_bir_lowering=False, debug=False, num_devices=B, enable_asserts=False)
    v = nc.declare_dram_parameter("value", [S, D], f32, isOutput=False)
    o = nc.declare_dram_parameter("out", [S, D], f32, isOutput=True)
    vf, of = v[:], o[:]

    with tile.TileContext(nc) as tcx:
        with (
            tcx.tile_pool(name="in", bufs=1) as in_pool,
            tcx.tile_pool(name="acc", bufs=1) as acc_pool,
            tcx.tile_pool(name="ones", bufs=1) as ones_pool,
            tcx.tile_pool(name="bcast", bufs=1) as bcast_pool,
            tcx.tile_pool(name="warm", bufs=1) as warm_pool,
            tcx.tile_pool(name="psum", bufs=1, space="PSUM") as psum_pool,
        ):
            # Preload the ACT table so the tail-time scalar.copy is cheap.
            warm = warm_pool.tile([P, 2], f32)
            nc.vector.memset(warm[:], 0.0)
            nc.scalar.copy(warm[:, 0:1], warm[:, 1:2])

            ones = ones_pool.tile([P, P], f32)
            nc.vector.memset(ones[:], 1.0)

            # ---- input: one 512 KB full-width DMA per slot (sync queue).
            tiles = []
            for k in range(N_SLOT):
                t = in_pool.tile([P, D], f32, tag=f"s{k}")
                nc.sync.dma_start(t[:], vf[k * P : (k + 1) * P])
                tiles.append(t)

            ps = psum_pool.tile([P, D], f32)

            def mm(moving, start, stop):
                for h in range(2):
                    nc.tensor.matmul(
                        ps[:, h * H : (h + 1) * H],
                        ones[:],
                        moving[:, h * H : (h + 1) * H],
                        start=start,
                        stop=stop,
                    )

            # PE slots (interleaved so the PE never backlogs; ~1.7us per
            # slot at a ~3.7us assigned-arrival cadence).
            for j, k in enumerate(PE_SLOTS):
                mm(tiles[k][:], start=(j == 0), stop=False)

            # DVE slots, chained into acc; the final slot is DVE's.
            dve = [k for k in range(N_SLOT) if k not in PE_SLOTS]
            acc = acc_pool.tile([P, D], f32)
            nc.vector.tensor_copy(acc[:], tiles[dve[0]][:])
            for k in dve[1:]:
                nc.vector.tensor_add(acc[:], acc[:], tiles[k][:])

            # Fold acc into PSUM: the only post-stream PE work.
            mm(acc, start=False, stop=True)

            # PSUM -> SBUF in parallel halves.  ACT takes bank A (its
            # stop-matmul retires ~0.4us before bank B's) and DVE takes
            # bank B; emitted in that order so they truly overlap.
            bc = bcast_pool.tile([P, D], f32)
            nc.scalar.copy(bc[:, 0:H], ps[:, 0:H])
            nc.vector.tensor_copy(bc[:, H:D], ps[:, H:D])

            # ---- output: broadcast writes, issued from the sync engine
            # (idle and ring-empty by now; the scalar engine is still
            # finishing its copy when the first write must be issued).
            o_re = of.rearrange("(i n p) m -> i p n m", i=N_OUT, n=OUT_REP, p=P)
            src = bc[:].unsqueeze(1).to_broadcast((P, OUT_REP, D))
            for i in range(N_OUT):
                nc.sync.dma_start(o_re[i], src)

    nc.compile()
    return nc


def _get_program():
    if "nc" not in _CACHE:
        _CACHE["nc"] = _build_program()
    return _CACHE["nc"]


def kernel(query: np.ndarray, value: np.ndarray) -> np.ndarray:
    from concourse.bass_utils import run_bass_kernel_spmd

    del query  # output is exactly independent of query (see module docstring)
    value = np.ascontiguousarray(value, dtype=np.float32)
    assert value.shape == (B, S, D)

    nc = _get_program()
    in_maps = [{"value": value[b]} for b in range(B)]
    try:
        res = run_bass_kernel_spmd(nc, in_maps, list(range(B)))
    except Exception:
        # The tunneled runtime occasionally surfaces a transient
        # NRT_EXEC_UNIT_UNRECOVERABLE on the first dispatch; retry once.
        import time

        time.sleep(2.0)
        res = run_bass_kernel_spmd(nc, in_maps, list(range(B)))
    return np.stack([res.results[b]["out"] for b in range(B)], axis=0)
